# revision 13
# baseline (speedup 1.0000x reference)
"""GCN layer kernel for Trainium2 (8 NeuronCores, SPMD).

out = relu( D^{-1/2} (A+I) D^{-1/2} x W^T + b )

Math restructure (per node i):
    agg[i] = sum_{(i,j) in E+self} coef_ij * x[j],  coef_ij = dinv_i * dinv_j
    out[i] = relu( agg[i] @ W^T + b )

Device plan per core (core owns 49 of the 392 padded 128-node src chunks,
assigned by LPT to balance edge-block counts):
  For each owned chunk k: stream the host-materialized edge rows x[dst]
  (bf16, pre-bucketed contiguous layout -> large HWDGE DMA descriptors,
  no gpsimd gather), build coef-valued one-hot selection matrices S on
  the DVE (two chunk-wide broadcast ops: (slot == iota) * coef),
  segment-reduce with PE matmuls accumulating in PSUM, transpose the
  [slot,256] sum on the PE, project through W^T (+bias via a K=1
  matmul), relu, store bf16.

Host does only sharding/layout work: degree counting, edge bucketing by
src chunk, materializing the gathered x rows into the per-core stream,
transposes/casts.  All FLOPs (segment sum, projection, relu) on device.
"""

import sys

for _p in ("/opt/trn_rl_repo",):
    if _p not in sys.path:
        sys.path.insert(0, _p)

from contextlib import ExitStack

import ml_dtypes
import numpy as np

import concourse.bass as bass
import concourse.mybir as mybir
import concourse.tile as tile
from concourse import bacc
from concourse.bass_utils import run_bass_kernel_spmd

BF16 = ml_dtypes.bfloat16

N_NODES = 50000
N_EDGES = 800000
F = 256  # in_size == out_size == 256
N_CORES = 8
NCH = (N_NODES + 127) // 128  # 391 real chunks of <=128 src nodes
CHUNKS = 49  # chunks per core (8*49 = 392 >= 391)
OUT_GRP = 8  # output chunks per DRAM write


def _build_program(nb_pos):
    """Build the (core-uniform) Bass program. nb_pos: per-position edge
    block counts (list of CHUNKS ints), shared across cores."""
    nc = bacc.Bacc(None, target_bir_lowering=False, debug=False)
    dt = mybir.dt

    nb_pos = [int(v) for v in nb_pos]
    nb2 = [v + (v & 1) for v in nb_pos]  # even-ified (local_scatter slices)
    totb = int(sum(nb_pos))
    totc = int(sum(nb2))
    b0x = np.concatenate([[0], np.cumsum(nb_pos)]).astype(np.int64)
    b0c = np.concatenate([[0], np.cumsum(nb2)]).astype(np.int64)

    xg = nc.dram_tensor("xg", [128, totb, F], dt.bfloat16, kind="ExternalInput")
    lsidx = nc.dram_tensor("lsidx", [128, totc], dt.int16, kind="ExternalInput")
    coef = nc.dram_tensor("coef", [128, totc], dt.bfloat16, kind="ExternalInput")
    wT = nc.dram_tensor("wt", [2, 128, F], dt.bfloat16, kind="ExternalInput")
    bias = nc.dram_tensor("bias", [128, F], dt.float32, kind="ExternalInput")
    ident = nc.dram_tensor("ident", [128, 128], dt.bfloat16, kind="ExternalInput")
    out = nc.dram_tensor("out", [CHUNKS * 128, F], dt.bfloat16, kind="ExternalOutput")

    with tile.TileContext(nc) as tc, ExitStack() as top:
        cpool = top.enter_context(tc.tile_pool(name="const", bufs=1))
        wt_s = cpool.tile([128, 2, F], dt.bfloat16)
        nc.sync.dma_start(out=wt_s[:, 0, :], in_=wT[0])
        nc.sync.dma_start(out=wt_s[:, 1, :], in_=wT[1])
        b_s = cpool.tile([128, F], dt.float32)
        nc.sync.dma_start(out=b_s[:], in_=bias[:])
        id_s = cpool.tile([128, 128], dt.bfloat16)
        nc.sync.dma_start(out=id_s[:], in_=ident[:])
        lsi_s = cpool.tile([128, totc], dt.int16)
        nc.sync.dma_start(out=lsi_s[:], in_=lsidx[:])
        cof_s = cpool.tile([128, totc], dt.bfloat16)
        nc.sync.dma_start(out=cof_s[:], in_=coef[:])

        with ExitStack() as p2:
            gpool = p2.enter_context(tc.tile_pool(name="gat", bufs=4))
            spool = p2.enter_context(tc.tile_pool(name="sel", bufs=4))
            apool = p2.enter_context(tc.tile_pool(name="agg", bufs=10))
            tpool = p2.enter_context(tc.tile_pool(name="aggT", bufs=10))
            opool = p2.enter_context(tc.tile_pool(name="ostg", bufs=3))
            xpool = p2.enter_context(tc.tile_pool(name="tmp", bufs=3))
            ps_p = p2.enter_context(tc.tile_pool(name="ps", bufs=4, space="PSUM"))
            pt_p = p2.enter_context(tc.tile_pool(name="pT", bufs=2, space="PSUM"))
            po_p = p2.enter_context(tc.tile_pool(name="po", bufs=2, space="PSUM"))

            st = {}

            def emit_scatter(k):
                NB = nb_pos[k]
                N2 = nb2[k]
                G = gpool.tile([128, NB, F], dt.bfloat16, tag="G")
                nc.sync.dma_start(
                    out=G[:], in_=xg[:, int(b0x[k]) : int(b0x[k]) + NB, :]
                )
                # S[e, b, slot] = (slot == slot_e) * coef_e on the gpsimd
                # engine: zero + sparse write of coef_e at (b%14)*128+slot.
                S = spool.tile([128, N2, 128], dt.bfloat16, tag="S")
                C0 = int(b0c[k])
                for g0 in range(0, N2, 14):
                    w = min(14, N2 - g0)
                    nc.gpsimd.local_scatter(
                        S[:, g0 : g0 + w, :],
                        cof_s[:, C0 + g0 : C0 + g0 + w],
                        lsi_s[:, C0 + g0 : C0 + g0 + w],
                        128,
                        w * 128,
                        w,
                    )
                ps = ps_p.tile([128, F], dt.float32)
                for b in range(NB):
                    nc.tensor.matmul(
                        out=ps[:],
                        lhsT=S[:, b, :],
                        rhs=G[:, b, :],
                        start=(b == 0),
                        stop=(b == NB - 1),
                    )
                agg = apool.tile([128, F], dt.bfloat16, tag="agg")
                nc.scalar.activation(
                    out=agg[:], in_=ps[:], func=mybir.ActivationFunctionType.Copy
                )
                st[k] = {"agg": agg}

            for g0 in range(0, CHUNKS, OUT_GRP):
                grp = list(range(g0, min(g0 + OUT_GRP, CHUNKS)))
                og = len(grp)
                # scatter + agg-copy for the whole group (PE runs NB matmuls
                # back-to-back per chunk; scalar/gpsimd/DMA chase in parallel)
                for k in grp:
                    emit_scatter(k)
                # PE transposes for the group (all agg deps long satisfied)
                for k in grp:
                    agg = st[k].pop("agg")
                    pT = pt_p.tile([128, 2, 128], dt.bfloat16)
                    for h in range(2):
                        nc.tensor.transpose(
                            pT[:, h, :], agg[:, h * 128 : (h + 1) * 128], id_s[:]
                        )
                    aggT = tpool.tile([128, 2, 128], dt.bfloat16, tag="aT")
                    nc.vector.tensor_copy(aggT[:], pT[:])
                    st[k]["aggT"] = aggT
                # PE projections for the group
                ob = opool.tile([128, og, F], dt.bfloat16, tag="ob")
                for k in grp:
                    aggT = st[k].pop("aggT")
                    po = po_p.tile([128, F], dt.float32)
                    nc.tensor.matmul(
                        out=po[:], lhsT=aggT[:, 0, :], rhs=wt_s[:, 0, :],
                        start=True, stop=False,
                    )
                    nc.tensor.matmul(
                        out=po[:], lhsT=aggT[:, 1, :], rhs=wt_s[:, 1, :],
                        start=False, stop=True,
                    )
                    tmp = xpool.tile([128, F], dt.float32, tag="tmp")
                    nc.vector.tensor_tensor(
                        out=tmp[:], in0=po[:], in1=b_s[:], op=mybir.AluOpType.add
                    )
                    nc.vector.tensor_scalar(
                        out=ob[:, k - g0, :],
                        in0=tmp[:],
                        scalar1=0.0,
                        scalar2=None,
                        op0=mybir.AluOpType.max,
                    )
                    del st[k]
                r0 = g0 * 128
                dst = out[r0 : r0 + og * 128, :].rearrange("(t p) f -> p t f", p=128)
                nc.sync.dma_start(out=dst, in_=ob[:])

    nc.compile()
    return nc


def _prep(x, edge_index, W, b):
    """Host-side sharding/layout. Returns (nb_pos, core_chunks, common,
    per_core)."""
    src = np.asarray(edge_index[0], dtype=np.int64)
    dst = np.asarray(edge_index[1], dtype=np.int64)
    deg = np.bincount(src, minlength=N_NODES).astype(np.float64)
    dinv = np.where(deg > 0, deg, 1.0) ** -0.5
    dinv[deg == 0] = 0.0

    loop = np.arange(N_NODES, dtype=np.int64)
    srcA = np.concatenate([src, loop])
    dstA = np.concatenate([dst, loop])
    coefA = (dinv[srcA] * dinv[dstA]).astype(np.float32)
    g = srcA >> 7
    slotA = (srcA & 127).astype(np.float32)

    nchp = N_CORES * CHUNKS  # 392 incl. one dummy chunk
    cnt = np.bincount(g, minlength=nchp)
    nbc = (cnt + 127) // 128

    # LPT assignment of chunks to cores, balancing total block count
    order_ch = np.argsort(-nbc, kind="stable")
    loads = np.zeros(N_CORES, dtype=np.int64)
    nassigned = np.zeros(N_CORES, dtype=np.int64)
    core_chunks = [[] for _ in range(N_CORES)]
    for ch in order_ch:
        cands = [c for c in range(N_CORES) if nassigned[c] < CHUNKS]
        c = min(cands, key=lambda cc: (loads[cc], cc))
        core_chunks[c].append(int(ch))
        loads[c] += nbc[ch]
        nassigned[c] += 1
    nb_pos = np.zeros(CHUNKS, dtype=np.int64)
    for c in range(N_CORES):
        for j, ch in enumerate(core_chunks[c]):
            nb_pos[j] = max(nb_pos[j], nbc[ch])
    nb_pos = np.maximum(nb_pos, 1)
    nb2 = nb_pos + (nb_pos & 1)
    b0x = np.concatenate([[0], np.cumsum(nb_pos)]).astype(np.int64)
    b0c = np.concatenate([[0], np.cumsum(nb2)]).astype(np.int64)
    totb = int(b0x[-1])
    totc = int(b0c[-1])

    eorder = np.argsort(g, kind="stable")
    seg_end = np.cumsum(cnt)
    seg_start = seg_end - cnt

    x_bf = np.asarray(x, dtype=np.float32).astype(BF16)
    wTf = np.ascontiguousarray(np.asarray(W, dtype=np.float32).T).astype(BF16)
    common = dict(
        wt=np.stack([wTf[:128], wTf[128:]]),
        bias=np.tile(np.asarray(b, dtype=np.float32)[None, :], (128, 1)),
        ident=np.eye(128, dtype=np.float32).astype(BF16),
    )

    per_core = []
    for c in range(N_CORES):
        xga = np.zeros((totb * 128, F), dtype=BF16)
        lsi = np.full((totc * 128,), -1, dtype=np.int64)
        cof = np.zeros((totc * 128,), dtype=np.float32)
        for j, ch in enumerate(core_chunks[c]):
            e = eorder[seg_start[ch] : seg_end[ch]]
            px = int(b0x[j]) * 128
            pc = int(b0c[j]) * 128
            xga[px : px + len(e)] = x_bf[dstA[e]]
            bloc = np.arange(len(e)) // 128  # block within chunk
            lsi[pc : pc + len(e)] = (bloc % 14) * 128 + srcA[e] % 128
            cof[pc : pc + len(e)] = coefA[e]
        per_core.append(
            dict(
                xg=np.ascontiguousarray(
                    xga.reshape(totb, 128, F).transpose(1, 0, 2)
                ),
                lsidx=np.ascontiguousarray(
                    lsi.reshape(totc, 128).T
                ).astype(np.int16),
                coef=np.ascontiguousarray(cof.reshape(totc, 128).T).astype(BF16),
            )
        )
    return nb_pos, core_chunks, common, per_core


def _install_ntff_hook():
    """The agent image's antenv lacks axon_hooks; recreate it so
    run_bass_kernel_spmd(trace=True) can profile via the axon .so."""
    import types

    if "antenv.axon_hooks" in sys.modules:
        return
    mod = types.ModuleType("antenv.axon_hooks")
    state = {}
    mod.set_axon_ntff_profile_hook = lambda h: state.__setitem__("h", h)
    mod.get_axon_ntff_profile_hook = lambda: state.get("h")
    sys.modules["antenv.axon_hooks"] = mod
    try:
        import antenv

        antenv.axon_hooks = mod
    except Exception:
        pass
    try:
        if "/root/.axon_site" not in sys.path:
            sys.path.insert(0, "/root/.axon_site")
        from trn_agent_boot.trn_boot import _ntff_profile_via_ctypes

        mod.set_axon_ntff_profile_hook(
            _ntff_profile_via_ctypes("/opt/axon/libaxon_pjrt.so")
        )
    except Exception:
        pass


_CACHE = {}


def kernel(x, edge_index, W, b, trace=False):
    if trace:
        _install_ntff_hook()
    nb_pos, core_chunks, common, per_core = _prep(x, edge_index, W, b)
    key = tuple(int(v) for v in nb_pos)
    if key not in _CACHE:
        _CACHE[key] = _build_program(nb_pos)
    nc = _CACHE[key]

    in_maps = []
    for c in range(N_CORES):
        m = dict(common)
        m.update(per_core[c])
        in_maps.append(m)

    res = run_bass_kernel_spmd(
        nc, in_maps, core_ids=list(range(N_CORES)), trace=trace
    )
    out_full = np.empty((N_NODES, F), dtype=np.float32)
    for c in range(N_CORES):
        oc = np.asarray(res.results[c]["out"], dtype=np.float32)
        for j, ch in enumerate(core_chunks[c]):
            if ch >= NCH:
                continue
            r0 = ch * 128
            r1 = min(r0 + 128, N_NODES)
            out_full[r0:r1] = oc[j * 128 : j * 128 + (r1 - r0)]
    if trace:
        kernel.last_exec_ns = res.exec_time_ns
        kernel.last_profile = res.profile_json
    return out_full


# revision 14
# speedup vs baseline: 1.0129x; 1.0129x over previous
"""GCN layer kernel for Trainium2 (8 NeuronCores, SPMD).

out = relu( D^{-1/2} (A+I) D^{-1/2} x W^T + b )

Math restructure (per node i):
    agg[i] = sum_{(i,j) in E+self} coef_ij * x[j],  coef_ij = dinv_i * dinv_j
    out[i] = relu( agg[i] @ W^T + b )

Device plan per core (core owns 49 of the 392 padded 128-node src chunks,
assigned by LPT to balance edge-block counts):
  For each owned chunk k: stream the host-materialized edge rows x[dst]
  (bf16, pre-bucketed contiguous layout -> large HWDGE DMA descriptors,
  no gpsimd gather), build coef-valued one-hot selection matrices S on
  the DVE (two chunk-wide broadcast ops: (slot == iota) * coef),
  segment-reduce with PE matmuls accumulating in PSUM, transpose the
  [slot,256] sum on the PE, project through W^T (+bias via a K=1
  matmul), relu, store bf16.

Host does only sharding/layout work: degree counting, edge bucketing by
src chunk, materializing the gathered x rows into the per-core stream,
transposes/casts.  All FLOPs (segment sum, projection, relu) on device.
"""

import sys

for _p in ("/opt/trn_rl_repo",):
    if _p not in sys.path:
        sys.path.insert(0, _p)

from contextlib import ExitStack

import ml_dtypes
import numpy as np

import concourse.bass as bass
import concourse.mybir as mybir
import concourse.tile as tile
from concourse import bacc
from concourse.bass_utils import run_bass_kernel_spmd

BF16 = ml_dtypes.bfloat16

N_NODES = 50000
N_EDGES = 800000
F = 256  # in_size == out_size == 256
N_CORES = 8
NCH = (N_NODES + 127) // 128  # 391 real chunks of <=128 src nodes
CHUNKS = 49  # chunks per core (8*49 = 392 >= 391)
OUT_GRP = 8  # output chunks per DRAM write


def _build_program(nb_pos):
    """Build the (core-uniform) Bass program. nb_pos: per-position edge
    block counts (list of CHUNKS ints), shared across cores."""
    nc = bacc.Bacc(None, target_bir_lowering=False, debug=False)
    dt = mybir.dt

    nb_pos = [int(v) for v in nb_pos]
    nb2 = [v + (v & 1) for v in nb_pos]  # even-ified (local_scatter slices)
    totb = int(sum(nb_pos))
    totc = int(sum(nb2))
    b0x = np.concatenate([[0], np.cumsum(nb_pos)]).astype(np.int64)
    b0c = np.concatenate([[0], np.cumsum(nb2)]).astype(np.int64)

    xg = nc.dram_tensor("xg", [128, totb, F], dt.bfloat16, kind="ExternalInput")
    lsidx = nc.dram_tensor("lsidx", [128, totc], dt.int16, kind="ExternalInput")
    coef = nc.dram_tensor("coef", [128, totc], dt.bfloat16, kind="ExternalInput")
    wT = nc.dram_tensor("wt", [2, 128, F], dt.bfloat16, kind="ExternalInput")
    bias = nc.dram_tensor("bias", [128, F], dt.float32, kind="ExternalInput")
    ident = nc.dram_tensor("ident", [128, 128], dt.bfloat16, kind="ExternalInput")
    out = nc.dram_tensor("out", [CHUNKS * 128, F], dt.bfloat16, kind="ExternalOutput")

    with tile.TileContext(nc) as tc, ExitStack() as top:
        cpool = top.enter_context(tc.tile_pool(name="const", bufs=1))
        wt_s = cpool.tile([128, 2, F], dt.bfloat16)
        nc.sync.dma_start(out=wt_s[:, 0, :], in_=wT[0])
        nc.sync.dma_start(out=wt_s[:, 1, :], in_=wT[1])
        b_s = cpool.tile([128, F], dt.float32)
        nc.sync.dma_start(out=b_s[:], in_=bias[:])
        id_s = cpool.tile([128, 128], dt.bfloat16)
        nc.sync.dma_start(out=id_s[:], in_=ident[:])
        lsi_s = cpool.tile([128, totc], dt.int16)
        nc.sync.dma_start(out=lsi_s[:], in_=lsidx[:])
        cof_s = cpool.tile([128, totc], dt.bfloat16)
        nc.sync.dma_start(out=cof_s[:], in_=coef[:])

        with ExitStack() as p2:
            gpool = p2.enter_context(tc.tile_pool(name="gat", bufs=7))
            spool = p2.enter_context(tc.tile_pool(name="sel", bufs=6))
            apool = p2.enter_context(tc.tile_pool(name="agg", bufs=10))
            tpool = p2.enter_context(tc.tile_pool(name="aggT", bufs=10))
            opool = p2.enter_context(tc.tile_pool(name="ostg", bufs=3))
            xpool = p2.enter_context(tc.tile_pool(name="tmp", bufs=3))
            ps_p = p2.enter_context(tc.tile_pool(name="ps", bufs=4, space="PSUM"))
            pt_p = p2.enter_context(tc.tile_pool(name="pT", bufs=2, space="PSUM"))
            po_p = p2.enter_context(tc.tile_pool(name="po", bufs=2, space="PSUM"))

            st = {}

            def emit_scatter(k):
                NB = nb_pos[k]
                N2 = nb2[k]
                G = gpool.tile([128, NB, F], dt.bfloat16, tag="G")
                nc.sync.dma_start(
                    out=G[:], in_=xg[:, int(b0x[k]) : int(b0x[k]) + NB, :]
                )
                # S[e, b, slot] = (slot == slot_e) * coef_e on the gpsimd
                # engine: zero + sparse write of coef_e at (b%14)*128+slot.
                S = spool.tile([128, N2, 128], dt.bfloat16, tag="S")
                C0 = int(b0c[k])
                for g0 in range(0, N2, 14):
                    w = min(14, N2 - g0)
                    nc.gpsimd.local_scatter(
                        S[:, g0 : g0 + w, :],
                        cof_s[:, C0 + g0 : C0 + g0 + w],
                        lsi_s[:, C0 + g0 : C0 + g0 + w],
                        128,
                        w * 128,
                        w,
                    )
                ps = ps_p.tile([128, F], dt.float32)
                for b in range(NB):
                    nc.tensor.matmul(
                        out=ps[:],
                        lhsT=S[:, b, :],
                        rhs=G[:, b, :],
                        start=(b == 0),
                        stop=(b == NB - 1),
                    )
                agg = apool.tile([128, F], dt.bfloat16, tag="agg")
                nc.scalar.activation(
                    out=agg[:], in_=ps[:], func=mybir.ActivationFunctionType.Copy
                )
                st[k] = {"agg": agg}

            for g0 in range(0, CHUNKS, OUT_GRP):
                grp = list(range(g0, min(g0 + OUT_GRP, CHUNKS)))
                og = len(grp)
                # scatter + agg-copy for the whole group (PE runs NB matmuls
                # back-to-back per chunk; scalar/gpsimd/DMA chase in parallel)
                for k in grp:
                    emit_scatter(k)
                # PE transposes for the group (all agg deps long satisfied)
                for k in grp:
                    agg = st[k].pop("agg")
                    pT = pt_p.tile([128, 2, 128], dt.bfloat16)
                    for h in range(2):
                        nc.tensor.transpose(
                            pT[:, h, :], agg[:, h * 128 : (h + 1) * 128], id_s[:]
                        )
                    aggT = tpool.tile([128, 2, 128], dt.bfloat16, tag="aT")
                    nc.vector.tensor_copy(aggT[:], pT[:])
                    st[k]["aggT"] = aggT
                # PE projections for the group
                ob = opool.tile([128, og, F], dt.bfloat16, tag="ob")
                for k in grp:
                    aggT = st[k].pop("aggT")
                    po = po_p.tile([128, F], dt.float32)
                    nc.tensor.matmul(
                        out=po[:], lhsT=aggT[:, 0, :], rhs=wt_s[:, 0, :],
                        start=True, stop=False,
                    )
                    nc.tensor.matmul(
                        out=po[:], lhsT=aggT[:, 1, :], rhs=wt_s[:, 1, :],
                        start=False, stop=True,
                    )
                    tmp = xpool.tile([128, F], dt.float32, tag="tmp")
                    nc.vector.tensor_tensor(
                        out=tmp[:], in0=po[:], in1=b_s[:], op=mybir.AluOpType.add
                    )
                    nc.vector.tensor_scalar(
                        out=ob[:, k - g0, :],
                        in0=tmp[:],
                        scalar1=0.0,
                        scalar2=None,
                        op0=mybir.AluOpType.max,
                    )
                    del st[k]
                r0 = g0 * 128
                dst = out[r0 : r0 + og * 128, :].rearrange("(t p) f -> p t f", p=128)
                nc.sync.dma_start(out=dst, in_=ob[:])

    nc.compile()
    return nc


def _prep(x, edge_index, W, b):
    """Host-side sharding/layout. Returns (nb_pos, core_chunks, common,
    per_core)."""
    src = np.asarray(edge_index[0], dtype=np.int64)
    dst = np.asarray(edge_index[1], dtype=np.int64)
    deg = np.bincount(src, minlength=N_NODES).astype(np.float64)
    dinv = np.where(deg > 0, deg, 1.0) ** -0.5
    dinv[deg == 0] = 0.0

    loop = np.arange(N_NODES, dtype=np.int64)
    srcA = np.concatenate([src, loop])
    dstA = np.concatenate([dst, loop])
    coefA = (dinv[srcA] * dinv[dstA]).astype(np.float32)
    g = srcA >> 7
    slotA = (srcA & 127).astype(np.float32)

    nchp = N_CORES * CHUNKS  # 392 incl. one dummy chunk
    cnt = np.bincount(g, minlength=nchp)
    nbc = (cnt + 127) // 128

    # LPT assignment of chunks to cores, balancing total block count
    order_ch = np.argsort(-nbc, kind="stable")
    loads = np.zeros(N_CORES, dtype=np.int64)
    nassigned = np.zeros(N_CORES, dtype=np.int64)
    core_chunks = [[] for _ in range(N_CORES)]
    for ch in order_ch:
        cands = [c for c in range(N_CORES) if nassigned[c] < CHUNKS]
        c = min(cands, key=lambda cc: (loads[cc], cc))
        core_chunks[c].append(int(ch))
        loads[c] += nbc[ch]
        nassigned[c] += 1
    nb_pos = np.zeros(CHUNKS, dtype=np.int64)
    for c in range(N_CORES):
        for j, ch in enumerate(core_chunks[c]):
            nb_pos[j] = max(nb_pos[j], nbc[ch])
    nb_pos = np.maximum(nb_pos, 1)
    nb2 = nb_pos + (nb_pos & 1)
    b0x = np.concatenate([[0], np.cumsum(nb_pos)]).astype(np.int64)
    b0c = np.concatenate([[0], np.cumsum(nb2)]).astype(np.int64)
    totb = int(b0x[-1])
    totc = int(b0c[-1])

    eorder = np.argsort(g, kind="stable")
    seg_end = np.cumsum(cnt)
    seg_start = seg_end - cnt

    x_bf = np.asarray(x, dtype=np.float32).astype(BF16)
    wTf = np.ascontiguousarray(np.asarray(W, dtype=np.float32).T).astype(BF16)
    common = dict(
        wt=np.stack([wTf[:128], wTf[128:]]),
        bias=np.tile(np.asarray(b, dtype=np.float32)[None, :], (128, 1)),
        ident=np.eye(128, dtype=np.float32).astype(BF16),
    )

    per_core = []
    for c in range(N_CORES):
        xga = np.zeros((totb * 128, F), dtype=BF16)
        lsi = np.full((totc * 128,), -1, dtype=np.int64)
        cof = np.zeros((totc * 128,), dtype=np.float32)
        for j, ch in enumerate(core_chunks[c]):
            e = eorder[seg_start[ch] : seg_end[ch]]
            px = int(b0x[j]) * 128
            pc = int(b0c[j]) * 128
            xga[px : px + len(e)] = x_bf[dstA[e]]
            bloc = np.arange(len(e)) // 128  # block within chunk
            lsi[pc : pc + len(e)] = (bloc % 14) * 128 + srcA[e] % 128
            cof[pc : pc + len(e)] = coefA[e]
        per_core.append(
            dict(
                xg=np.ascontiguousarray(
                    xga.reshape(totb, 128, F).transpose(1, 0, 2)
                ),
                lsidx=np.ascontiguousarray(
                    lsi.reshape(totc, 128).T
                ).astype(np.int16),
                coef=np.ascontiguousarray(cof.reshape(totc, 128).T).astype(BF16),
            )
        )
    return nb_pos, core_chunks, common, per_core


def _install_ntff_hook():
    """The agent image's antenv lacks axon_hooks; recreate it so
    run_bass_kernel_spmd(trace=True) can profile via the axon .so."""
    import types

    if "antenv.axon_hooks" in sys.modules:
        return
    mod = types.ModuleType("antenv.axon_hooks")
    state = {}
    mod.set_axon_ntff_profile_hook = lambda h: state.__setitem__("h", h)
    mod.get_axon_ntff_profile_hook = lambda: state.get("h")
    sys.modules["antenv.axon_hooks"] = mod
    try:
        import antenv

        antenv.axon_hooks = mod
    except Exception:
        pass
    try:
        if "/root/.axon_site" not in sys.path:
            sys.path.insert(0, "/root/.axon_site")
        from trn_agent_boot.trn_boot import _ntff_profile_via_ctypes

        mod.set_axon_ntff_profile_hook(
            _ntff_profile_via_ctypes("/opt/axon/libaxon_pjrt.so")
        )
    except Exception:
        pass


_CACHE = {}


def kernel(x, edge_index, W, b, trace=False):
    if trace:
        _install_ntff_hook()
    nb_pos, core_chunks, common, per_core = _prep(x, edge_index, W, b)
    key = tuple(int(v) for v in nb_pos)
    if key not in _CACHE:
        _CACHE[key] = _build_program(nb_pos)
    nc = _CACHE[key]

    in_maps = []
    for c in range(N_CORES):
        m = dict(common)
        m.update(per_core[c])
        in_maps.append(m)

    res = run_bass_kernel_spmd(
        nc, in_maps, core_ids=list(range(N_CORES)), trace=trace
    )
    out_full = np.empty((N_NODES, F), dtype=np.float32)
    for c in range(N_CORES):
        oc = np.asarray(res.results[c]["out"], dtype=np.float32)
        for j, ch in enumerate(core_chunks[c]):
            if ch >= NCH:
                continue
            r0 = ch * 128
            r1 = min(r0 + 128, N_NODES)
            out_full[r0:r1] = oc[j * 128 : j * 128 + (r1 - r0)]
    if trace:
        kernel.last_exec_ns = res.exec_time_ns
        kernel.last_profile = res.profile_json
    return out_full
